# revision 8
# baseline (speedup 1.0000x reference)
"""AttentionRNN Trainium2 kernel.

Reference computation (per batch element b):
    xp[t] = x[b,t] @ Wx.T + b_ih                     (Wx = W_ih[:, :256])
    h[t]  = tanh(xp[t] + h[t-1] @ Wh.T)              (Wh = W_ih[:, 256:])
    scores[s] = <h[s], h[S-1]>;  attn = softmax(scores)
    ctx = sum_s attn[s] h[s];    out[b] = ctx @ W_ho.T + b_ho

Sharding: data-parallel, batch 64 -> 8 cores x 8.

Per-core design (fp32 PSUM accumulation):
  - Weights transposed once on PE. Wh is stored as fp8 e3m4 scaled by
    S=128 (Xavier weights would be subnormal in raw e3m4); the scan's
    tanh ACT applies scale=1/S. fp8 weights halve the per-step
    LDWEIGHTS stream (FWL reads 4 elem/cycle) which is the PE-side
    floor of the sequential scan.
  - xp archive holds S*(x@Wx.T + b_ih) in fp16.
  - Scan step t: identity-matmul seeds PSUM[128,4x8] with S*xp for a
    group of GS steps, 16 (LDW+MM) pairs accumulate (S*Wh.T)@h chunks,
    then ONE ACT tanh (scale=1/S) writes all 4 m-chunks of h[t] (fp16).
    Critical path per step = PE->ACT handoff + ACT + ACT->PE handoff.
  - The x load / PE transpose / xp projection for block i+1 is
    interleaved instruction-by-instruction into block i's scan so it
    runs inside the scan's idle PE/DVE windows.
  - Attention scores: per (sblk,b,k) matmuls into column b of a PSUM
    tile (engines cannot cross partitions; DMA cannot read PSUM).
  - Softmax on [8, 1024] with fused exp+sum (accum_out).
  - Context: per-(b,kc,s-block) DVE multiply with accum_out partial
    sums, then one reduce over s-blocks.
"""
import numpy as np
from contextlib import ExitStack

import concourse.bacc as bacc
import concourse.tile as tile
from concourse import mybir
from concourse.masks import make_identity

F32 = mybir.dt.float32
F16 = mybir.dt.float16
F8 = mybir.dt.float8e3          # e3m4: 4 mantissa bits, max ~15.9
AF = mybir.ActivationFunctionType
ALU = mybir.AluOpType

B, S, I, H, O = 64, 1024, 256, 512, 256
NCORES = 8
BL = B // NCORES          # 8 batch elements per core
NB = S // 128             # 8 time blocks
KI = I // 128             # 2 input k-chunks
KH = H // 128             # 4 hidden k-chunks
MO = O // 128             # 2 output chunks
WSCALE = 128.0            # fp8 weight pre-scale


def build_nc(seq_blocks=NB, reps=1, fp8=True):
    nb = seq_blocks
    s_len = nb * 128
    wdt = F8 if fp8 else F16
    wsc = WSCALE if fp8 else 1.0
    nc = bacc.Bacc("TRN2", target_bir_lowering=False, debug=False,
                   num_devices=NCORES)
    x_s = nc.dram_tensor("x_s", [BL, s_len, I], F32, kind="ExternalInput").ap()
    w_ih = nc.dram_tensor("w_ih", [H, I + H], F32, kind="ExternalInput").ap()
    b_ih = nc.dram_tensor("b_ih", [H], F32, kind="ExternalInput").ap()
    w_ho = nc.dram_tensor("w_ho", [O, H], F32, kind="ExternalInput").ap()
    b_ho = nc.dram_tensor("b_ho", [O], F32, kind="ExternalInput").ap()
    out_s = nc.dram_tensor("out_s", [BL, O], F32, kind="ExternalOutput").ap()

    with ExitStack() as ctx:
        tc = ctx.enter_context(tile.TileContext(nc))
        perm = ctx.enter_context(tc.tile_pool(name="perm", bufs=1))

        id32 = perm.tile([128, 128], F32, tag="id32")
        id16 = perm.tile([128, 128], F16, tag="id16")
        make_identity(nc, id32)
        make_identity(nc, id16)

        # witx[p, k, m, j] = Wx[m*128+j, k*128+p]      (fp16)
        # wh8[p, k, m, j]  = WSCALE*Wh[m*128+j, k*128+p] (fp8 e3m4)
        witx = perm.tile([128, KI, KH, 128], F16, tag="witx")
        wh8 = perm.tile([128, KH, KH, 128], wdt, tag="wh8")
        # whoT[p, k, c, j] = W_ho[c*128+j, k*128+p]
        whoT = perm.tile([128, KH, MO, 128], F16, tag="whoT")
        bih = perm.tile([128, KH], F32, tag="bih")
        bho = perm.tile([128, MO], F32, tag="bho")

        nc.sync.dma_start(out=bih, in_=b_ih.rearrange("(m p) -> p m", p=128))
        nc.sync.dma_start(out=bho, in_=b_ho.rearrange("(c p) -> p c", p=128))

        # ---- P0: load + transpose weights --------------------------------
        with tc.tile_pool(name="wstage", bufs=1) as wstage, \
             tc.tile_pool(name="ps_tr", bufs=3, space="PSUM") as ps_tr:
            w_nat = wstage.tile([128, KH, I + H], F32, tag="w_nat")
            nc.sync.dma_start(
                out=w_nat, in_=w_ih.rearrange("(c p) j -> p c j", p=128))
            who_nat = wstage.tile([128, MO, H], F32, tag="who_nat")
            nc.sync.dma_start(
                out=who_nat, in_=w_ho.rearrange("(c p) j -> p c j", p=128))

            for c in range(KH):
                for k in range(KI):
                    tr = ps_tr.tile([128, 128], F32)
                    nc.tensor.transpose(
                        tr, w_nat[:, c, k * 128:(k + 1) * 128], id32)
                    nc.vector.tensor_copy(out=witx[:, k, c, :], in_=tr)
                for k in range(KH):
                    tr = ps_tr.tile([128, 128], F32)
                    nc.tensor.transpose(
                        tr, w_nat[:, c, (KI + k) * 128:(KI + k + 1) * 128],
                        id32)
                    nc.vector.tensor_scalar_mul(
                        out=wh8[:, k, c, :], in0=tr, scalar1=wsc)
            for c in range(MO):
                for k in range(KH):
                    tr = ps_tr.tile([128, 128], F32)
                    nc.tensor.transpose(
                        tr, who_nat[:, c, k * 128:(k + 1) * 128], id32)
                    nc.vector.tensor_copy(out=whoT[:, k, c, :], in_=tr)

        # Per-time-block archives.
        xT_pool = ctx.enter_context(tc.tile_pool(name="xT", bufs=1))
        xpT_pool = ctx.enter_context(tc.tile_pool(name="xpT", bufs=1))
        hs_pool = ctx.enter_context(tc.tile_pool(name="hs", bufs=1))
        # xT[p, c, tt, b] = x[b, blk*128+tt, c*128+p]
        xT = [xT_pool.tile([128, KI, 128, BL], F16, name=f"xT{i}", tag=f"xT{i}")
              for i in range(nb)]
        # xpT[p, m, tt, b] = WSCALE*xp[blk*128+tt, b, m*128+p]
        xpT = [xpT_pool.tile([128, KH, 128, BL], F16, name=f"xpT{i}", tag=f"xpT{i}")
               for i in range(nb)]
        # hs[p, m, tt, b] = h[blk*128+tt][b, m*128+p]
        hs = [hs_pool.tile([128, KH, 128, BL], F16, name=f"hs{i}", tag=f"hs{i}")
              for i in range(nb)]

        for rep in range(reps):
            with tc.tile_pool(name="xstage", bufs=4) as xstage, \
                 tc.tile_pool(name="ps_tr2", bufs=2, space="PSUM") as ps_tr2, \
                 tc.tile_pool(name="ps_xp", bufs=2, space="PSUM") as ps_xp, \
                 tc.tile_pool(name="ps_scan", bufs=4, space="PSUM") as ps_scan:

                # -- head work for one block, as a list of emit-closures ----
                def head_items(blk):
                    items = []
                    t0 = blk * 128
                    xst = [None] * BL

                    def dma(b):
                        def go():
                            xst[b] = xstage.tile([128, I], F32, name="xst")
                            nc.sync.dma_start(out=xst[b],
                                              in_=x_s[b, t0:t0 + 128, :])
                        return go

                    def trcopy(b, c):
                        def go():
                            tr = ps_tr2.tile([128, 128], F32)
                            nc.tensor.transpose(
                                tr, xst[b][:, c * 128:(c + 1) * 128], id32)
                            nc.vector.tensor_copy(out=xT[blk][:, c, :, b],
                                                  in_=tr)
                        return go

                    pxp = [None] * (KH * 2)

                    def xpmm(m, half, k):
                        def go():
                            if k == 0:
                                pxp[m * 2 + half] = ps_xp.tile(
                                    [128, 512], F32, name="pxp")
                            tsl = slice(half * 64, (half + 1) * 64)
                            nc.tensor.matmul(
                                pxp[m * 2 + half], witx[:, k, m, :],
                                xT[blk][:, k, tsl, :],
                                start=(k == 0), stop=(k == KI - 1))
                        return go

                    def xpbias(m, half):
                        def go():
                            tsl = slice(half * 64, (half + 1) * 64)
                            nc.vector.tensor_scalar(
                                out=xpT[blk][:, m, tsl, :],
                                in0=pxp[m * 2 + half].rearrange(
                                    "p (t b) -> p t b", b=BL),
                                scalar1=bih[:, m:m + 1], scalar2=wsc,
                                op0=ALU.add, op1=ALU.mult)
                        return go

                    for b in range(BL):
                        items.append(dma(b))
                    for b in range(BL):
                        for c in range(KI):
                            items.append(trcopy(b, c))
                    for m in range(KH):
                        for half in range(2):
                            for k in range(KI):
                                items.append(xpmm(m, half, k))
                            items.append(xpbias(m, half))
                    return items

                def run_items(items):
                    for it in items:
                        it()

                # prologue: block 0's head runs un-interleaved
                run_items(head_items(0))

                # -- scan with next block's head interleaved ----------------
                GS = 4
                for blk in range(nb):
                    nxt = head_items(blk + 1) if blk + 1 < nb else []
                    # spread: dma early, transposes from step 48, xp from 96
                    sched = {}
                    for i in range(BL):
                        sched.setdefault(2 * i, []).append(nxt[i]) if nxt \
                            else None
                    for i in range(BL * KI):
                        if nxt:
                            sched.setdefault(48 + 2 * i, []).append(nxt[BL + i])
                    rest = nxt[BL + BL * KI:]
                    for i, it in enumerate(rest):
                        sched.setdefault(96 + i, []).append(it)

                    for off in range(0, 128, GS):
                        g0 = blk * 128 + off
                        ps = ps_scan.tile([128, KH, GS, BL], F32)
                        nc.tensor.matmul(ps, id16,
                                         xpT[blk][:, :, off:off + GS, :],
                                         start=True, stop=False,
                                         skip_group_check=True)
                        for j in range(GS):
                            t = g0 + j
                            if t > 0:
                                pblk, poff = (t - 1) // 128, (t - 1) % 128
                                for m in range(KH):
                                    for k in range(KH):
                                        nc.tensor.matmul(
                                            ps[:, m, j, :], wh8[:, k, m, :],
                                            hs[pblk][:, k, poff, :],
                                            start=False,
                                            stop=(j == GS - 1 and m == KH - 1
                                                  and k == KH - 1),
                                            skip_group_check=True)
                            nc.scalar.activation(
                                out=hs[blk][:, :, off + j, :],
                                in_=ps[:, :, j, :], func=AF.Tanh,
                                scale=1.0 / wsc)
                            for it in sched.get(off + j, []):
                                it()

            scores = perm.tile([8, s_len], F32, tag="scores")
            hf_blk, hf_off = nb - 1, 127

            # ---- P4: attention scores ----------------------------------------
            with tc.tile_pool(name="ps_sc", bufs=4, space="PSUM") as ps_sc, \
                 tc.tile_pool(name="ps_st", bufs=2, space="PSUM") as ps_st, \
                 tc.tile_pool(name="scst", bufs=2) as scst:
                for sblk in range(nb):
                    psc = ps_sc.tile([128, BL], F32, tag="psc")
                    first = True
                    for b in range(BL):
                        for k in range(KH):
                            nc.tensor.matmul(
                                psc[:, b:b + 1],
                                hs[sblk][:, k, :, b],
                                hs[hf_blk][:, k, hf_off, b:b + 1],
                                start=first,
                                stop=(b == BL - 1 and k == KH - 1))
                            first = False
                    st = scst.tile([128, BL], F32, tag="st")
                    nc.vector.tensor_copy(out=st, in_=psc)
                    ptr = ps_st.tile([8, 128], F32, tag="ptr")
                    nc.tensor.transpose(ptr, st, id32)
                    nc.vector.tensor_copy(
                        out=scores[:, sblk * 128:(sblk + 1) * 128], in_=ptr)

            # ---- P5: softmax --------------------------------------------------
            negmax = perm.tile([8, 1], F32, tag="negmax")
            sumexp = perm.tile([8, 1], F32, tag="sumexp")
            recip = perm.tile([8, 1], F32, tag="recip")
            p_sb = perm.tile([8, s_len], F32, tag="p_sb")
            attn16 = perm.tile([8, s_len], F16, tag="attn16")
            nc.vector.tensor_reduce(out=negmax, in_=scores,
                                    axis=mybir.AxisListType.X, op=ALU.max,
                                    negate=True)
            nc.scalar.activation(out=p_sb, in_=scores, func=AF.Exp,
                                 bias=negmax, scale=1.0, accum_out=sumexp)
            nc.vector.reciprocal(recip, sumexp)
            nc.vector.tensor_scalar_mul(attn16, p_sb, recip)

            # ---- P6+P7: context = sum_s attn[s] * h[s] ------------------------
            ctxparts = perm.tile([128, KH, BL, nb], F32, tag="ctxparts")
            ctx32 = perm.tile([128, KH, BL], F32, tag="ctx32")
            ctx16 = perm.tile([128, KH, BL], F16, tag="ctx16")
            with tc.tile_pool(name="dram", bufs=1, space="DRAM") as dpool, \
                 tc.tile_pool(name="bcp", bufs=2) as bcp, \
                 tc.tile_pool(name="tmpp", bufs=2) as tmpp:
                attn_d = dpool.tile([8, s_len], F16, name=f"attn_d_r{rep}")
                nc.sync.dma_start(out=attn_d, in_=attn16)
                for b in range(BL):
                    bc = bcp.tile([128, s_len], F16)
                    nc.sync.dma_start(
                        out=bc, in_=attn_d[b:b + 1, :].to_broadcast((128, s_len)))
                    for kc in range(KH):
                        for blk in range(nb):
                            eng = nc.vector if (kc + blk) % 2 == 0 else nc.gpsimd
                            tmp = tmpp.tile([128, 128], F16, name="ctmp")
                            eng.scalar_tensor_tensor(
                                out=tmp, in0=hs[blk][:, kc, :, b], scalar=1.0,
                                in1=bc[:, blk * 128:(blk + 1) * 128],
                                op0=ALU.mult, op1=ALU.mult,
                                accum_out=ctxparts[:, kc, b, blk:blk + 1])
            nc.vector.tensor_reduce(out=ctx32, in_=ctxparts,
                                    axis=mybir.AxisListType.X, op=ALU.add)
            nc.vector.tensor_copy(out=ctx16, in_=ctx32)

            # ---- P8: output projection + transpose back ----------------------
            out_sb = perm.tile([8, O], F32, tag="out_sb")
            outT32 = perm.tile([128, MO, BL], F32, tag="outT32")
            with tc.tile_pool(name="ps_pr", bufs=1, space="PSUM") as ps_pr, \
                 tc.tile_pool(name="ps_tr3", bufs=2, space="PSUM") as ps_tr3:
                po = ps_pr.tile([128, MO, BL], F32)
                first = True
                for k in range(KH):
                    for c in range(MO):
                        nc.tensor.matmul(po[:, c, :], whoT[:, k, c, :],
                                         ctx16[:, k, :], start=first,
                                         stop=(k == KH - 1 and c == MO - 1))
                        first = False
                for c in range(MO):
                    nc.vector.tensor_scalar_add(out=outT32[:, c, :],
                                                in0=po[:, c, :],
                                                scalar1=bho[:, c:c + 1])
                for c in range(MO):
                    tr = ps_tr3.tile([8, 128], F32)
                    nc.tensor.transpose(tr, outT32[:, c, :], id32)
                    nc.vector.tensor_copy(out=out_sb[:, c * 128:(c + 1) * 128],
                                          in_=tr)
            nc.sync.dma_start(out=out_s[:, :], in_=out_sb)

    nc.compile()
    return nc


_CACHE = {}


def kernel(x, W_ih, b_ih, W_ho, b_ho):
    from concourse.bass_utils import run_bass_kernel_spmd
    if "nc" not in _CACHE:
        _CACHE["nc"] = build_nc()
    x = np.ascontiguousarray(np.asarray(x, np.float32))
    in_maps = []
    for c in range(NCORES):
        in_maps.append({
            "x_s": x[c * BL:(c + 1) * BL],
            "w_ih": np.asarray(W_ih, np.float32),
            "b_ih": np.asarray(b_ih, np.float32),
            "w_ho": np.asarray(W_ho, np.float32),
            "b_ho": np.asarray(b_ho, np.float32),
        })
    res = run_bass_kernel_spmd(_CACHE["nc"], in_maps,
                               list(range(NCORES))).results
    return np.concatenate([res[c]["out_s"] for c in range(NCORES)], axis=0)


# revision 18
# speedup vs baseline: 1.1804x; 1.1804x over previous
"""AttentionRNN Trainium2 kernel.

Reference computation (per batch element b):
    xp[t] = x[b,t] @ Wx.T + b_ih                     (Wx = W_ih[:, :256])
    h[t]  = tanh(xp[t] + h[t-1] @ Wh.T)              (Wh = W_ih[:, 256:])
    scores[s] = <h[s], h[S-1]>;  attn = softmax(scores)
    ctx = sum_s attn[s] h[s];    out[b] = ctx @ W_ho.T + b_ho

Sharding: data-parallel, batch 64 -> 8 cores x 8.

Per-core design (fp32 PSUM accumulation):
  - Weights transposed once on PE. Wh is stored as fp8 e3m4 scaled by
    S=128 (Xavier weights would be subnormal in raw e3m4); the scan's
    tanh ACT applies scale=1/S. fp8 weights halve the per-step
    LDWEIGHTS stream (FWL reads 4 elem/cycle) which is the PE-side
    floor of the sequential scan.
  - xp archive holds S*(x@Wx.T + b_ih) in fp16.
  - Scan step t: identity-matmul seeds PSUM[128,4x8] with S*xp for a
    group of GS steps, 16 (LDW+MM) pairs accumulate (S*Wh.T)@h chunks,
    then ONE ACT tanh (scale=1/S) writes all 4 m-chunks of h[t] (fp16).
    Critical path per step = PE->ACT handoff + ACT + ACT->PE handoff.
  - The x load / PE transpose / xp projection for block i+1 is
    interleaved instruction-by-instruction into block i's scan so it
    runs inside the scan's idle PE/DVE windows.
  - Attention scores: per (sblk,b,k) matmuls into column b of a PSUM
    tile (engines cannot cross partitions; DMA cannot read PSUM).
  - Softmax on [8, 1024] with fused exp+sum (accum_out).
  - Context: per-(b,kc,s-block) DVE multiply with accum_out partial
    sums, then one reduce over s-blocks.
"""
import numpy as np
from contextlib import ExitStack

import concourse.bacc as bacc
import concourse.tile as tile
from concourse import mybir
from concourse.masks import make_identity

F32 = mybir.dt.float32
F16 = mybir.dt.float16
F8 = mybir.dt.float8e3          # e3m4: 4 mantissa bits, max ~15.9
AF = mybir.ActivationFunctionType
ALU = mybir.AluOpType

B, S, I, H, O = 64, 1024, 256, 512, 256
NCORES = 8
BL = B // NCORES          # 8 batch elements per core
NB = S // 128             # 8 time blocks
KI = I // 128             # 2 input k-chunks
KH = H // 128             # 4 hidden k-chunks
MO = O // 128             # 2 output chunks
WSCALE = 128.0            # fp8 weight pre-scale


def build_nc(seq_blocks=NB, reps=1, fp8=False, use_gpsimd=False):
    nb = seq_blocks
    s_len = nb * 128
    wdt = F8 if fp8 else F16
    wsc = WSCALE if fp8 else 1.0
    nc = bacc.Bacc("TRN2", target_bir_lowering=False, debug=False,
                   num_devices=NCORES)
    x_s = nc.dram_tensor("x_s", [BL, s_len, I], F32, kind="ExternalInput").ap()
    w_ih = nc.dram_tensor("w_ih", [H, I + H], F32, kind="ExternalInput").ap()
    b_ih = nc.dram_tensor("b_ih", [H], F32, kind="ExternalInput").ap()
    w_ho = nc.dram_tensor("w_ho", [O, H], F32, kind="ExternalInput").ap()
    b_ho = nc.dram_tensor("b_ho", [O], F32, kind="ExternalInput").ap()
    out_s = nc.dram_tensor("out_s", [BL, O], F32, kind="ExternalOutput").ap()

    with ExitStack() as ctx:
        tc = ctx.enter_context(tile.TileContext(nc))
        perm = ctx.enter_context(tc.tile_pool(name="perm", bufs=1))

        id32 = perm.tile([128, 128], F32, tag="id32")
        id16 = perm.tile([128, 128], F16, tag="id16")
        make_identity(nc, id32)
        make_identity(nc, id16)

        # witx[p, k, m, j] = Wx[m*128+j, k*128+p]      (fp16)
        # wh8[p, k, m, j]  = WSCALE*Wh[m*128+j, k*128+p] (fp8 e3m4)
        witx = perm.tile([128, KI, KH, 128], F16, tag="witx")
        wh8 = perm.tile([128, KH, KH, 128], wdt, tag="wh8")
        # whoT[p, k, c, j] = W_ho[c*128+j, k*128+p]
        whoT = perm.tile([128, KH, MO, 128], F16, tag="whoT")
        bih = perm.tile([128, KH], F32, tag="bih")
        bho = perm.tile([128, MO], F32, tag="bho")

        nc.sync.dma_start(out=bih, in_=b_ih.rearrange("(m p) -> p m", p=128))
        nc.sync.dma_start(out=bho, in_=b_ho.rearrange("(c p) -> p c", p=128))

        # ---- P0: load + transpose weights --------------------------------
        with tc.tile_pool(name="wstage", bufs=1) as wstage, \
             tc.tile_pool(name="ps_tr", bufs=3, space="PSUM") as ps_tr:
            w_nat = wstage.tile([128, KH, I + H], F32, tag="w_nat")
            nc.sync.dma_start(
                out=w_nat, in_=w_ih.rearrange("(c p) j -> p c j", p=128))
            who_nat = wstage.tile([128, MO, H], F32, tag="who_nat")
            nc.sync.dma_start(
                out=who_nat, in_=w_ho.rearrange("(c p) j -> p c j", p=128))

            for c in range(KH):
                for k in range(KI):
                    tr = ps_tr.tile([128, 128], F32)
                    nc.tensor.transpose(
                        tr, w_nat[:, c, k * 128:(k + 1) * 128], id32)
                    nc.vector.tensor_copy(out=witx[:, k, c, :], in_=tr)
                for k in range(KH):
                    tr = ps_tr.tile([128, 128], F32)
                    nc.tensor.transpose(
                        tr, w_nat[:, c, (KI + k) * 128:(KI + k + 1) * 128],
                        id32)
                    nc.vector.tensor_scalar_mul(
                        out=wh8[:, k, c, :], in0=tr, scalar1=wsc)
            for c in range(MO):
                for k in range(KH):
                    tr = ps_tr.tile([128, 128], F32)
                    nc.tensor.transpose(
                        tr, who_nat[:, c, k * 128:(k + 1) * 128], id32)
                    nc.vector.tensor_copy(out=whoT[:, k, c, :], in_=tr)

        # Per-time-block archives.
        xT_pool = ctx.enter_context(tc.tile_pool(name="xT", bufs=1))
        xpT_pool = ctx.enter_context(tc.tile_pool(name="xpT", bufs=1))
        hs_pool = ctx.enter_context(tc.tile_pool(name="hs", bufs=1))
        # xT[p, c, tt, b] = x[b, blk*128+tt, c*128+p]
        xT = [xT_pool.tile([128, KI, 128, BL], F16, name=f"xT{i}", tag=f"xT{i}")
              for i in range(nb)]
        # xpT[p, m, tt, b] = WSCALE*xp[blk*128+tt, b, m*128+p]
        xpT = [xpT_pool.tile([128, KH, 128, BL], F16, name=f"xpT{i}", tag=f"xpT{i}")
               for i in range(nb)]
        # hs[p, m, tt, b] = h[blk*128+tt][b, m*128+p]
        hs = [hs_pool.tile([128, KH, 128, BL], F16, name=f"hs{i}", tag=f"hs{i}")
              for i in range(nb)]

        for rep in range(reps):
            with tc.tile_pool(name="xstage", bufs=4) as xstage, \
                 tc.tile_pool(name="ps_tr2", bufs=2, space="PSUM") as ps_tr2, \
                 tc.tile_pool(name="ps_xp", bufs=2, space="PSUM") as ps_xp, \
                 tc.tile_pool(name="ps_scan", bufs=4, space="PSUM") as ps_scan:

                # -- head work for one block, as a list of emit-closures ----
                def head_items(blk):
                    items = []
                    t0 = blk * 128
                    xst = [None] * BL

                    def dma(b):
                        def go():
                            xst[b] = xstage.tile([128, I], F32, name="xst")
                            nc.sync.dma_start(out=xst[b],
                                              in_=x_s[b, t0:t0 + 128, :])
                        return go

                    def trcopy(b, c):
                        def go():
                            tr = ps_tr2.tile([128, 128], F32)
                            nc.tensor.transpose(
                                tr, xst[b][:, c * 128:(c + 1) * 128], id32)
                            nc.vector.tensor_copy(out=xT[blk][:, c, :, b],
                                                  in_=tr)
                        return go

                    pxp = [None] * (KH * 2)

                    def xpmm(m, half, k):
                        def go():
                            if k == 0:
                                pxp[m * 2 + half] = ps_xp.tile(
                                    [128, 512], F32, name="pxp")
                            tsl = slice(half * 64, (half + 1) * 64)
                            nc.tensor.matmul(
                                pxp[m * 2 + half], witx[:, k, m, :],
                                xT[blk][:, k, tsl, :],
                                start=(k == 0), stop=(k == KI - 1))
                        return go

                    def xpbias(m, half):
                        def go():
                            tsl = slice(half * 64, (half + 1) * 64)
                            nc.vector.tensor_scalar(
                                out=xpT[blk][:, m, tsl, :],
                                in0=pxp[m * 2 + half].rearrange(
                                    "p (t b) -> p t b", b=BL),
                                scalar1=bih[:, m:m + 1], scalar2=wsc,
                                op0=ALU.add, op1=ALU.mult)
                        return go

                    for b in range(BL):
                        items.append(dma(b))
                    for b in range(BL):
                        for c in range(KI):
                            items.append(trcopy(b, c))
                    for m in range(KH):
                        for half in range(2):
                            for k in range(KI):
                                items.append(xpmm(m, half, k))
                            items.append(xpbias(m, half))
                    return items

                def run_items(items):
                    for it in items:
                        it()

                # prologue: block 0's head runs un-interleaved
                run_items(head_items(0))

                # -- scan with next block's head interleaved ----------------
                GS = 4
                for blk in range(nb):
                    nxt = head_items(blk + 1) if blk + 1 < nb else []
                    # spread: dma early, transposes from step 48, xp from 96
                    sched = {}
                    for i in range(BL):
                        sched.setdefault(2 * i, []).append(nxt[i]) if nxt \
                            else None
                    for i in range(BL * KI):
                        if nxt:
                            sched.setdefault(48 + 2 * i, []).append(nxt[BL + i])
                    rest = nxt[BL + BL * KI:]
                    for i, it in enumerate(rest):
                        sched.setdefault(96 + i, []).append(it)

                    for off in range(0, 128, GS):
                        g0 = blk * 128 + off
                        ps = ps_scan.tile([128, KH, GS, BL], F32)
                        nc.tensor.matmul(ps, id16,
                                         xpT[blk][:, :, off:off + GS, :],
                                         start=True, stop=False,
                                         skip_group_check=True)
                        for j in range(GS):
                            t = g0 + j
                            if t > 0:
                                pblk, poff = (t - 1) // 128, (t - 1) % 128
                                for m in range(KH):
                                    for k in range(KH):
                                        nc.tensor.matmul(
                                            ps[:, m, j, :], wh8[:, k, m, :],
                                            hs[pblk][:, k, poff, :],
                                            start=False,
                                            stop=(j == GS - 1 and m == KH - 1
                                                  and k == KH - 1),
                                            skip_group_check=True)
                            nc.scalar.activation(
                                out=hs[blk][:, :, off + j, :],
                                in_=ps[:, :, j, :], func=AF.Tanh,
                                scale=1.0 / wsc)
                            for it in sched.get(off + j, []):
                                it()

            scores = perm.tile([8, s_len], F32, tag="scores")
            hf_blk, hf_off = nb - 1, 127

            # ---- P4: attention scores ----------------------------------------
            with tc.tile_pool(name="ps_sc", bufs=4, space="PSUM") as ps_sc, \
                 tc.tile_pool(name="ps_st", bufs=2, space="PSUM") as ps_st, \
                 tc.tile_pool(name="scst", bufs=2) as scst:
                for sblk in range(nb):
                    psc = ps_sc.tile([128, BL], F32, tag="psc")
                    first = True
                    for b in range(BL):
                        for k in range(KH):
                            nc.tensor.matmul(
                                psc[:, b:b + 1],
                                hs[sblk][:, k, :, b],
                                hs[hf_blk][:, k, hf_off, b:b + 1],
                                start=first,
                                stop=(b == BL - 1 and k == KH - 1))
                            first = False
                    st = scst.tile([128, BL], F32, tag="st")
                    nc.vector.tensor_copy(out=st, in_=psc)
                    ptr = ps_st.tile([8, 128], F32, tag="ptr")
                    nc.tensor.transpose(ptr, st, id32)
                    nc.vector.tensor_copy(
                        out=scores[:, sblk * 128:(sblk + 1) * 128], in_=ptr)

            # ---- P5: softmax --------------------------------------------------
            negmax = perm.tile([8, 1], F32, tag="negmax")
            sumexp = perm.tile([8, 1], F32, tag="sumexp")
            recip = perm.tile([8, 1], F32, tag="recip")
            p_sb = perm.tile([8, s_len], F32, tag="p_sb")
            attn16 = perm.tile([8, s_len], F16, tag="attn16")
            nc.vector.tensor_reduce(out=negmax, in_=scores,
                                    axis=mybir.AxisListType.X, op=ALU.max,
                                    negate=True)
            nc.scalar.activation(out=p_sb, in_=scores, func=AF.Exp,
                                 bias=negmax, scale=1.0, accum_out=sumexp)
            nc.vector.reciprocal(recip, sumexp)
            nc.vector.tensor_scalar_mul(attn16, p_sb, recip)

            # ---- P6+P7: context = sum_s attn[s] * h[s] ------------------------
            ctxparts = perm.tile([128, KH, BL, nb], F32, tag="ctxparts")
            ctx32 = perm.tile([128, KH, BL], F32, tag="ctx32")
            ctx16 = perm.tile([128, KH, BL], F16, tag="ctx16")
            with tc.tile_pool(name="dram", bufs=1, space="DRAM") as dpool, \
                 tc.tile_pool(name="bcp", bufs=2) as bcp, \
                 tc.tile_pool(name="tmpp", bufs=4) as tmpp:
                attn_d = dpool.tile([8, s_len], F16, name=f"attn_d_r{rep}")
                nc.sync.dma_start(out=attn_d, in_=attn16)
                for b in range(BL):
                    bc = bcp.tile([128, s_len], F16)
                    nc.sync.dma_start(
                        out=bc, in_=attn_d[b:b + 1, :].to_broadcast((128, s_len)))
                    for kc in range(KH):
                        for blk in range(nb):
                            dve = (not use_gpsimd) or (kc + blk) % 2 == 0
                            eng = nc.vector if dve else nc.gpsimd
                            tmp = tmpp.tile([128, 128], F16,
                                            name="ctmpv" if dve else "ctmpp")
                            eng.scalar_tensor_tensor(
                                out=tmp, in0=hs[blk][:, kc, :, b], scalar=1.0,
                                in1=bc[:, blk * 128:(blk + 1) * 128],
                                op0=ALU.mult, op1=ALU.mult,
                                accum_out=ctxparts[:, kc, b, blk:blk + 1])
            nc.vector.tensor_reduce(out=ctx32, in_=ctxparts,
                                    axis=mybir.AxisListType.X, op=ALU.add)
            nc.vector.tensor_copy(out=ctx16, in_=ctx32)

            # ---- P8: output projection + transpose back ----------------------
            out_sb = perm.tile([8, O], F32, tag="out_sb")
            outT32 = perm.tile([128, MO, BL], F32, tag="outT32")
            with tc.tile_pool(name="ps_pr", bufs=1, space="PSUM") as ps_pr, \
                 tc.tile_pool(name="ps_tr3", bufs=2, space="PSUM") as ps_tr3:
                po = ps_pr.tile([128, MO, BL], F32)
                first = True
                for k in range(KH):
                    for c in range(MO):
                        nc.tensor.matmul(po[:, c, :], whoT[:, k, c, :],
                                         ctx16[:, k, :], start=first,
                                         stop=(k == KH - 1 and c == MO - 1))
                        first = False
                for c in range(MO):
                    nc.vector.tensor_scalar_add(out=outT32[:, c, :],
                                                in0=po[:, c, :],
                                                scalar1=bho[:, c:c + 1])
                for c in range(MO):
                    tr = ps_tr3.tile([8, 128], F32)
                    nc.tensor.transpose(tr, outT32[:, c, :], id32)
                    nc.vector.tensor_copy(out=out_sb[:, c * 128:(c + 1) * 128],
                                          in_=tr)
            nc.sync.dma_start(out=out_s[:, :], in_=out_sb)

    nc.compile()
    return nc


_CACHE = {}


def _make_runner():
    """Compile once; return a callable(in_maps) -> list of per-core outs.
    Mirrors concourse.bass2jax.run_bass_via_pjrt but caches the jitted
    executable so repeated kernel() calls skip retrace/recompile."""
    import jax
    from jax.sharding import Mesh, PartitionSpec
    from jax.experimental.shard_map import shard_map
    from concourse import mybir as mb
    from concourse.bass2jax import (_bass_exec_p, install_neuronx_cc_hook,
                                    partition_id_tensor)

    nc = build_nc()
    install_neuronx_cc_hook()
    partition_name = (
        nc.partition_id_tensor.name if nc.partition_id_tensor else None)
    in_names, out_names, out_avals, zero_outs = [], [], [], []
    for alloc in nc.m.functions[0].allocations:
        if not isinstance(alloc, mb.MemoryLocationSet):
            continue
        name = alloc.memorylocations[0].name
        if alloc.kind == "ExternalInput":
            if name != partition_name:
                in_names.append(name)
        elif alloc.kind == "ExternalOutput":
            shape = tuple(alloc.tensor_shape)
            dtype = mb.dt.np(alloc.dtype)
            out_avals.append(jax.core.ShapedArray(shape, dtype))
            out_names.append(name)
            zero_outs.append(np.zeros(shape, dtype))
    if nc.dbg_addr is not None:
        in_names.append(nc.dbg_addr.name)
    n_params = len(in_names)
    all_in_names = list(in_names) + list(out_names)
    if partition_name is not None:
        all_in_names.append(partition_name)
    n_outs = len(out_avals)
    donate = tuple(range(n_params, n_params + n_outs))

    def _body(*args):
        operands = list(args)
        if partition_name is not None:
            operands.append(partition_id_tensor())
        outs = _bass_exec_p.bind(
            *operands, out_avals=tuple(out_avals),
            in_names=tuple(all_in_names), out_names=tuple(out_names),
            lowering_input_output_aliases=(),
            sim_require_finite=True, sim_require_nnan=True, nc=nc)
        return tuple(outs)

    devices = jax.devices()[:NCORES]
    mesh = Mesh(np.asarray(devices), ("core",))
    in_specs = (PartitionSpec("core"),) * (n_params + n_outs)
    out_specs = (PartitionSpec("core"),) * n_outs
    sharded = jax.jit(
        shard_map(_body, mesh=mesh, in_specs=in_specs,
                  out_specs=out_specs, check_rep=False),
        donate_argnums=donate, keep_unused=True)

    def run(in_maps):
        per_core = []
        for m in in_maps:
            if nc.dbg_addr is not None:
                m = {**m, nc.dbg_addr.name: np.zeros((1, 2), np.uint32)}
            per_core.append([np.asarray(m[n]) for n in in_names])
        concat_in = [
            np.concatenate([per_core[c][i] for c in range(NCORES)], axis=0)
            for i in range(n_params)]
        concat_zeros = [
            np.zeros((NCORES * z.shape[0], *z.shape[1:]), z.dtype)
            for z in zero_outs]
        out_arrs = sharded(*concat_in, *concat_zeros)
        return [
            {name: np.asarray(out_arrs[i]).reshape(
                NCORES, *out_avals[i].shape)[c]
             for i, name in enumerate(out_names)}
            for c in range(NCORES)]

    return run


def kernel(x, W_ih, b_ih, W_ho, b_ho):
    if "runner" not in _CACHE:
        _CACHE["runner"] = _make_runner()
    x = np.ascontiguousarray(np.asarray(x, np.float32))
    in_maps = []
    for c in range(NCORES):
        in_maps.append({
            "x_s": np.ascontiguousarray(x[c * BL:(c + 1) * BL]),
            "w_ih": np.asarray(W_ih, np.float32),
            "b_ih": np.asarray(b_ih, np.float32),
            "w_ho": np.asarray(W_ho, np.float32),
            "b_ho": np.asarray(b_ho, np.float32),
        })
    res = _CACHE["runner"](in_maps)
    return np.concatenate([res[c]["out_s"] for c in range(NCORES)], axis=0)


# revision 22
# speedup vs baseline: 1.9478x; 1.6501x over previous
"""AttentionRNN Trainium2 kernel.

Reference computation (per batch element b):
    xp[t] = x[b,t] @ Wx.T + b_ih                     (Wx = W_ih[:, :256])
    h[t]  = tanh(xp[t] + h[t-1] @ Wh.T)              (Wh = W_ih[:, 256:])
    scores[s] = <h[s], h[S-1]>;  attn = softmax(scores)
    ctx = sum_s attn[s] h[s];    out[b] = ctx @ W_ho.T + b_ho

Sharding: data-parallel, batch 64 -> 8 cores x 8.

Per-core design (fp16 operands, fp32 PSUM accumulation):
  - Weights transposed once on PE, stored fp16 (fp8=True switches Wh
    to e3m4 scaled by S=128 with tanh scale=1/S; measured only ~30us
    faster because this bass/walrus path has no FWL, so LDWEIGHTS cost
    is dtype-independent — not worth the 25x accuracy loss).
  - Scan step t: identity-matmul seeds PSUM[128,4x8] with xp for a
    group of GS=8 steps, 16 (LDW+MM N=8) pairs accumulate Wh.T@h
    chunks, then ONE ACT tanh writes all 4 m-chunks of h[t] (fp16) in
    a single strided instruction. On HW the step floor is the 16-pair
    LDW stream (~78ns/pair, no FWL); the ACT chain hides under it.
  - The x load / PE transpose / xp projection for block i+1 is
    interleaved instruction-by-instruction into block i's scan so it
    runs inside the scan's idle PE/DVE/DMA windows (head 117us -> 30us).
  - Attention scores: per (sblk,b,k) matmuls into column b of a PSUM
    tile (engines cannot cross partitions; DMA cannot read PSUM).
  - Softmax on [8, 1024] with fused exp+sum (accum_out).
  - Context: per-(b,kc,s-block) DVE multiply with accum_out partial
    sums, then one reduce over s-blocks.
"""
import numpy as np
from contextlib import ExitStack

import concourse.bacc as bacc
import concourse.tile as tile
from concourse import mybir
from concourse.masks import make_identity

F32 = mybir.dt.float32
F16 = mybir.dt.float16
F8 = mybir.dt.float8e3          # e3m4: 4 mantissa bits, max ~15.9
AF = mybir.ActivationFunctionType
ALU = mybir.AluOpType

B, S, I, H, O = 64, 1024, 256, 512, 256
NCORES = 8
BL = B // NCORES          # 8 batch elements per core
NB = S // 128             # 8 time blocks
KI = I // 128             # 2 input k-chunks
KH = H // 128             # 4 hidden k-chunks
MO = O // 128             # 2 output chunks
WSCALE = 128.0            # fp8 weight pre-scale


def build_nc(seq_blocks=NB, reps=1, fp8=False, use_gpsimd=False):
    nb = seq_blocks
    s_len = nb * 128
    wdt = F8 if fp8 else F16
    wsc = WSCALE if fp8 else 1.0
    nc = bacc.Bacc("TRN2", target_bir_lowering=False, debug=False,
                   num_devices=NCORES)
    x_s = nc.dram_tensor("x_s", [BL, s_len, I], F32, kind="ExternalInput").ap()
    w_ih = nc.dram_tensor("w_ih", [H, I + H], F32, kind="ExternalInput").ap()
    b_ih = nc.dram_tensor("b_ih", [H], F32, kind="ExternalInput").ap()
    w_ho = nc.dram_tensor("w_ho", [O, H], F32, kind="ExternalInput").ap()
    b_ho = nc.dram_tensor("b_ho", [O], F32, kind="ExternalInput").ap()
    out_s = nc.dram_tensor("out_s", [BL, O], F32, kind="ExternalOutput").ap()

    with ExitStack() as ctx:
        tc = ctx.enter_context(tile.TileContext(nc))
        perm = ctx.enter_context(tc.tile_pool(name="perm", bufs=1))

        id32 = perm.tile([128, 128], F32, tag="id32")
        id16 = perm.tile([128, 128], F16, tag="id16")
        make_identity(nc, id32)
        make_identity(nc, id16)

        # witx[p, k, m, j] = Wx[m*128+j, k*128+p]      (fp16)
        # wh8[p, k, m, j]  = WSCALE*Wh[m*128+j, k*128+p] (fp8 e3m4)
        witx = perm.tile([128, KI, KH, 128], F16, tag="witx")
        wh8 = perm.tile([128, KH, KH, 128], wdt, tag="wh8")
        # whoT[p, k, c, j] = W_ho[c*128+j, k*128+p]
        whoT = perm.tile([128, KH, MO, 128], F16, tag="whoT")
        bih = perm.tile([128, KH], F32, tag="bih")
        bho = perm.tile([128, MO], F32, tag="bho")

        nc.sync.dma_start(out=bih, in_=b_ih.rearrange("(m p) -> p m", p=128))
        nc.sync.dma_start(out=bho, in_=b_ho.rearrange("(c p) -> p c", p=128))

        # ---- P0: load + transpose weights --------------------------------
        with tc.tile_pool(name="wstage", bufs=1) as wstage, \
             tc.tile_pool(name="ps_tr", bufs=3, space="PSUM") as ps_tr:
            w_nat = wstage.tile([128, KH, I + H], F32, tag="w_nat")
            nc.sync.dma_start(
                out=w_nat, in_=w_ih.rearrange("(c p) j -> p c j", p=128))
            who_nat = wstage.tile([128, MO, H], F32, tag="who_nat")
            nc.sync.dma_start(
                out=who_nat, in_=w_ho.rearrange("(c p) j -> p c j", p=128))

            for c in range(KH):
                for k in range(KI):
                    tr = ps_tr.tile([128, 128], F32)
                    nc.tensor.transpose(
                        tr, w_nat[:, c, k * 128:(k + 1) * 128], id32)
                    nc.vector.tensor_copy(out=witx[:, k, c, :], in_=tr)
                for k in range(KH):
                    tr = ps_tr.tile([128, 128], F32)
                    nc.tensor.transpose(
                        tr, w_nat[:, c, (KI + k) * 128:(KI + k + 1) * 128],
                        id32)
                    nc.vector.tensor_scalar_mul(
                        out=wh8[:, k, c, :], in0=tr, scalar1=wsc)
            for c in range(MO):
                for k in range(KH):
                    tr = ps_tr.tile([128, 128], F32)
                    nc.tensor.transpose(
                        tr, who_nat[:, c, k * 128:(k + 1) * 128], id32)
                    nc.vector.tensor_copy(out=whoT[:, k, c, :], in_=tr)

        # Per-time-block archives.
        xT_pool = ctx.enter_context(tc.tile_pool(name="xT", bufs=1))
        xpT_pool = ctx.enter_context(tc.tile_pool(name="xpT", bufs=1))
        hs_pool = ctx.enter_context(tc.tile_pool(name="hs", bufs=1))
        # xT[p, c, tt, b] = x[b, blk*128+tt, c*128+p]
        xT = [xT_pool.tile([128, KI, 128, BL], F16, name=f"xT{i}", tag=f"xT{i}")
              for i in range(nb)]
        # xpT[p, m, tt, b] = WSCALE*xp[blk*128+tt, b, m*128+p]
        xpT = [xpT_pool.tile([128, KH, 128, BL], F16, name=f"xpT{i}", tag=f"xpT{i}")
               for i in range(nb)]
        # hs[p, m, tt, b] = h[blk*128+tt][b, m*128+p]
        hs = [hs_pool.tile([128, KH, 128, BL], F16, name=f"hs{i}", tag=f"hs{i}")
              for i in range(nb)]

        for rep in range(reps):
            with tc.tile_pool(name="xstage", bufs=4) as xstage, \
                 tc.tile_pool(name="ps_tr2", bufs=2, space="PSUM") as ps_tr2, \
                 tc.tile_pool(name="ps_xp", bufs=2, space="PSUM") as ps_xp, \
                 tc.tile_pool(name="ps_scan", bufs=4, space="PSUM") as ps_scan:

                # -- head work for one block, as a list of emit-closures ----
                def head_items(blk):
                    items = []
                    t0 = blk * 128
                    xst = [None] * BL

                    def dma(b):
                        def go():
                            xst[b] = xstage.tile([128, I], F32, name="xst")
                            nc.sync.dma_start(out=xst[b],
                                              in_=x_s[b, t0:t0 + 128, :])
                        return go

                    def trcopy(b, c):
                        def go():
                            tr = ps_tr2.tile([128, 128], F32)
                            nc.tensor.transpose(
                                tr, xst[b][:, c * 128:(c + 1) * 128], id32)
                            nc.vector.tensor_copy(out=xT[blk][:, c, :, b],
                                                  in_=tr)
                        return go

                    pxp = [None] * (KH * 2)

                    def xpmm(m, half, k):
                        def go():
                            if k == 0:
                                pxp[m * 2 + half] = ps_xp.tile(
                                    [128, 512], F32, name="pxp")
                            tsl = slice(half * 64, (half + 1) * 64)
                            nc.tensor.matmul(
                                pxp[m * 2 + half], witx[:, k, m, :],
                                xT[blk][:, k, tsl, :],
                                start=(k == 0), stop=(k == KI - 1))
                        return go

                    def xpbias(m, half):
                        def go():
                            tsl = slice(half * 64, (half + 1) * 64)
                            nc.vector.tensor_scalar(
                                out=xpT[blk][:, m, tsl, :],
                                in0=pxp[m * 2 + half].rearrange(
                                    "p (t b) -> p t b", b=BL),
                                scalar1=bih[:, m:m + 1], scalar2=wsc,
                                op0=ALU.add, op1=ALU.mult)
                        return go

                    for b in range(BL):
                        items.append(dma(b))
                    for b in range(BL):
                        for c in range(KI):
                            items.append(trcopy(b, c))
                    for m in range(KH):
                        for half in range(2):
                            for k in range(KI):
                                items.append(xpmm(m, half, k))
                            items.append(xpbias(m, half))
                    return items

                def run_items(items):
                    for it in items:
                        it()

                # prologue: block 0's head runs un-interleaved
                run_items(head_items(0))

                # -- scan with next block's head interleaved ----------------
                GS = 8
                for blk in range(nb):
                    nxt = head_items(blk + 1) if blk + 1 < nb else []
                    # spread: dma early, transposes from step 48, xp from 96
                    sched = {}
                    for i in range(BL):
                        sched.setdefault(2 * i, []).append(nxt[i]) if nxt \
                            else None
                    for i in range(BL * KI):
                        if nxt:
                            sched.setdefault(48 + 2 * i, []).append(nxt[BL + i])
                    rest = nxt[BL + BL * KI:]
                    for i, it in enumerate(rest):
                        sched.setdefault(96 + i, []).append(it)

                    for off in range(0, 128, GS):
                        g0 = blk * 128 + off
                        ps = ps_scan.tile([128, KH, GS, BL], F32)
                        nc.tensor.matmul(ps, id16,
                                         xpT[blk][:, :, off:off + GS, :],
                                         start=True, stop=False,
                                         skip_group_check=True)
                        for j in range(GS):
                            t = g0 + j
                            if t > 0:
                                pblk, poff = (t - 1) // 128, (t - 1) % 128
                                for m in range(KH):
                                    for k in range(KH):
                                        nc.tensor.matmul(
                                            ps[:, m, j, :], wh8[:, k, m, :],
                                            hs[pblk][:, k, poff, :],
                                            start=False,
                                            stop=(j == GS - 1 and m == KH - 1
                                                  and k == KH - 1),
                                            skip_group_check=True)
                            nc.scalar.activation(
                                out=hs[blk][:, :, off + j, :],
                                in_=ps[:, :, j, :], func=AF.Tanh,
                                scale=1.0 / wsc)
                            for it in sched.get(off + j, []):
                                it()

            scores = perm.tile([8, s_len], F32, tag="scores")
            hf_blk, hf_off = nb - 1, 127

            # ---- P4: attention scores ----------------------------------------
            with tc.tile_pool(name="ps_sc", bufs=4, space="PSUM") as ps_sc, \
                 tc.tile_pool(name="ps_st", bufs=2, space="PSUM") as ps_st, \
                 tc.tile_pool(name="scst", bufs=2) as scst:
                for sblk in range(nb):
                    psc = ps_sc.tile([128, BL], F32, tag="psc")
                    first = True
                    for b in range(BL):
                        for k in range(KH):
                            nc.tensor.matmul(
                                psc[:, b:b + 1],
                                hs[sblk][:, k, :, b],
                                hs[hf_blk][:, k, hf_off, b:b + 1],
                                start=first,
                                stop=(b == BL - 1 and k == KH - 1))
                            first = False
                    st = scst.tile([128, BL], F32, tag="st")
                    nc.vector.tensor_copy(out=st, in_=psc)
                    ptr = ps_st.tile([8, 128], F32, tag="ptr")
                    nc.tensor.transpose(ptr, st, id32)
                    nc.vector.tensor_copy(
                        out=scores[:, sblk * 128:(sblk + 1) * 128], in_=ptr)

            # ---- P5: softmax --------------------------------------------------
            negmax = perm.tile([8, 1], F32, tag="negmax")
            sumexp = perm.tile([8, 1], F32, tag="sumexp")
            recip = perm.tile([8, 1], F32, tag="recip")
            p_sb = perm.tile([8, s_len], F32, tag="p_sb")
            attn16 = perm.tile([8, s_len], F16, tag="attn16")
            nc.vector.tensor_reduce(out=negmax, in_=scores,
                                    axis=mybir.AxisListType.X, op=ALU.max,
                                    negate=True)
            nc.scalar.activation(out=p_sb, in_=scores, func=AF.Exp,
                                 bias=negmax, scale=1.0, accum_out=sumexp)
            nc.vector.reciprocal(recip, sumexp)
            nc.vector.tensor_scalar_mul(attn16, p_sb, recip)

            # ---- P6+P7: context = sum_s attn[s] * h[s] ------------------------
            ctxparts = perm.tile([128, KH, BL, nb], F32, tag="ctxparts")
            ctx32 = perm.tile([128, KH, BL], F32, tag="ctx32")
            ctx16 = perm.tile([128, KH, BL], F16, tag="ctx16")
            with tc.tile_pool(name="dram", bufs=1, space="DRAM") as dpool, \
                 tc.tile_pool(name="bcp", bufs=2) as bcp, \
                 tc.tile_pool(name="tmpp", bufs=4) as tmpp:
                attn_d = dpool.tile([8, s_len], F16, name=f"attn_d_r{rep}")
                nc.sync.dma_start(out=attn_d, in_=attn16)
                for b in range(BL):
                    bc = bcp.tile([128, s_len], F16)
                    nc.sync.dma_start(
                        out=bc, in_=attn_d[b:b + 1, :].to_broadcast((128, s_len)))
                    for kc in range(KH):
                        for blk in range(nb):
                            tmp = tmpp.tile([128, 128], F16, name="ctmpv")
                            nc.vector.scalar_tensor_tensor(
                                out=tmp, in0=hs[blk][:, kc, :, b], scalar=1.0,
                                in1=bc[:, blk * 128:(blk + 1) * 128],
                                op0=ALU.mult, op1=ALU.mult,
                                accum_out=ctxparts[:, kc, b, blk:blk + 1])
            nc.vector.tensor_reduce(out=ctx32, in_=ctxparts,
                                    axis=mybir.AxisListType.X, op=ALU.add)
            nc.vector.tensor_copy(out=ctx16, in_=ctx32)

            # ---- P8: output projection + transpose back ----------------------
            out_sb = perm.tile([8, O], F32, tag="out_sb")
            outT32 = perm.tile([128, MO, BL], F32, tag="outT32")
            with tc.tile_pool(name="ps_pr", bufs=1, space="PSUM") as ps_pr, \
                 tc.tile_pool(name="ps_tr3", bufs=2, space="PSUM") as ps_tr3:
                po = ps_pr.tile([128, MO, BL], F32)
                first = True
                for k in range(KH):
                    for c in range(MO):
                        nc.tensor.matmul(po[:, c, :], whoT[:, k, c, :],
                                         ctx16[:, k, :], start=first,
                                         stop=(k == KH - 1 and c == MO - 1))
                        first = False
                for c in range(MO):
                    nc.vector.tensor_scalar_add(out=outT32[:, c, :],
                                                in0=po[:, c, :],
                                                scalar1=bho[:, c:c + 1])
                for c in range(MO):
                    tr = ps_tr3.tile([8, 128], F32)
                    nc.tensor.transpose(tr, outT32[:, c, :], id32)
                    nc.vector.tensor_copy(out=out_sb[:, c * 128:(c + 1) * 128],
                                          in_=tr)
            nc.sync.dma_start(out=out_s[:, :], in_=out_sb)

    nc.compile()
    return nc


_CACHE = {}


def _make_runner():
    """Compile once; return a callable(in_maps) -> list of per-core outs.
    Mirrors concourse.bass2jax.run_bass_via_pjrt but caches the jitted
    executable so repeated kernel() calls skip retrace/recompile."""
    import jax
    from jax.sharding import Mesh, PartitionSpec
    from jax.experimental.shard_map import shard_map
    from concourse import mybir as mb
    from concourse.bass2jax import (_bass_exec_p, install_neuronx_cc_hook,
                                    partition_id_tensor)

    nc = build_nc()
    install_neuronx_cc_hook()
    partition_name = (
        nc.partition_id_tensor.name if nc.partition_id_tensor else None)
    in_names, out_names, out_avals, zero_outs = [], [], [], []
    for alloc in nc.m.functions[0].allocations:
        if not isinstance(alloc, mb.MemoryLocationSet):
            continue
        name = alloc.memorylocations[0].name
        if alloc.kind == "ExternalInput":
            if name != partition_name:
                in_names.append(name)
        elif alloc.kind == "ExternalOutput":
            shape = tuple(alloc.tensor_shape)
            dtype = mb.dt.np(alloc.dtype)
            out_avals.append(jax.core.ShapedArray(shape, dtype))
            out_names.append(name)
            zero_outs.append(np.zeros(shape, dtype))
    if nc.dbg_addr is not None:
        in_names.append(nc.dbg_addr.name)
    n_params = len(in_names)
    all_in_names = list(in_names) + list(out_names)
    if partition_name is not None:
        all_in_names.append(partition_name)
    n_outs = len(out_avals)
    donate = tuple(range(n_params, n_params + n_outs))

    def _body(*args):
        operands = list(args)
        if partition_name is not None:
            operands.append(partition_id_tensor())
        outs = _bass_exec_p.bind(
            *operands, out_avals=tuple(out_avals),
            in_names=tuple(all_in_names), out_names=tuple(out_names),
            lowering_input_output_aliases=(),
            sim_require_finite=True, sim_require_nnan=True, nc=nc)
        return tuple(outs)

    devices = jax.devices()[:NCORES]
    mesh = Mesh(np.asarray(devices), ("core",))
    in_specs = (PartitionSpec("core"),) * (n_params + n_outs)
    out_specs = (PartitionSpec("core"),) * n_outs
    sharded = jax.jit(
        shard_map(_body, mesh=mesh, in_specs=in_specs,
                  out_specs=out_specs, check_rep=False),
        donate_argnums=donate, keep_unused=True)

    def run(in_maps):
        per_core = []
        for m in in_maps:
            if nc.dbg_addr is not None:
                m = {**m, nc.dbg_addr.name: np.zeros((1, 2), np.uint32)}
            per_core.append([np.asarray(m[n]) for n in in_names])
        concat_in = [
            np.concatenate([per_core[c][i] for c in range(NCORES)], axis=0)
            for i in range(n_params)]
        concat_zeros = [
            np.zeros((NCORES * z.shape[0], *z.shape[1:]), z.dtype)
            for z in zero_outs]
        out_arrs = sharded(*concat_in, *concat_zeros)
        return [
            {name: np.asarray(out_arrs[i]).reshape(
                NCORES, *out_avals[i].shape)[c]
             for i, name in enumerate(out_names)}
            for c in range(NCORES)]

    return run


def kernel(x, W_ih, b_ih, W_ho, b_ho):
    if "runner" not in _CACHE:
        _CACHE["runner"] = _make_runner()
    x = np.ascontiguousarray(np.asarray(x, np.float32))
    in_maps = []
    for c in range(NCORES):
        in_maps.append({
            "x_s": np.ascontiguousarray(x[c * BL:(c + 1) * BL]),
            "w_ih": np.asarray(W_ih, np.float32),
            "b_ih": np.asarray(b_ih, np.float32),
            "w_ho": np.asarray(W_ho, np.float32),
            "b_ho": np.asarray(b_ho, np.float32),
        })
    res = _CACHE["runner"](in_maps)
    return np.concatenate([res[c]["out_s"] for c in range(NCORES)], axis=0)
